# revision 1
# baseline (speedup 1.0000x reference)
"""Chunkwise causal attention (full causal MHA + QKV/out projections) on 8 trn2 cores.

v2: all-f16 datapath (f32 PSUM accumulation), batched DMAs (weights loaded once),
narrowed diagonal QK chunks, shared PSUM pools across phases.

Sharding: data-parallel over batch (B=2) x tensor-parallel over heads (16 -> 4 per
core). Host sums the 4 partial out-projections per batch and adds bout.

Self-contained: hardcodes all shapes from the problem spec.
"""

import numpy as np

import concourse.bass as bass
import concourse.mybir as mybir
import concourse.tile as tile
from concourse import bacc
from concourse.masks import make_identity

# Problem shapes
B, S, D = 2, 2048, 2048
H, Dh = 16, 128
HC = 4                      # heads per core
P = 128
SPLIT = 4
SQ = S // SPLIT             # 512 queries per outer phase
N_DC = D // P               # 16 contraction chunks for projections
N_SC = S // P               # 16 key chunks
SCALE = 1.0 / float(np.sqrt(Dh))
VW = 132                    # vaug row width (129 used: 128 dh + 1 ones col)

f32 = mybir.dt.float32
f16 = mybir.dt.float16

_COMPILED = {}

CFG = dict(XB=2, QB=2, VB=2, EXPB=20, SPB=4, OSB=3, BIGPS=5, PVB=2, TRB=1)


def build_program():
    nc = bacc.Bacc("TRN2", target_bir_lowering=False, debug=False)

    xp = nc.dram_tensor("xp", (P, SPLIT, N_DC, SQ), f16, kind="ExternalInput")
    wqkv = nc.dram_tensor("wqkv", (P, 3, HC, N_DC, P), f16, kind="ExternalInput")
    wout = nc.dram_tensor("wout", (P, HC, D), f16, kind="ExternalInput")
    bq = nc.dram_tensor("bq", (P, HC), f32, kind="ExternalInput")
    bk = nc.dram_tensor("bk", (P, HC), f32, kind="ExternalInput")
    bvb = nc.dram_tensor("bvb", (P, HC * Dh), f32, kind="ExternalInput")
    masks = nc.dram_tensor("masks", (4, P, 512), f16, kind="ExternalInput")
    outp = nc.dram_tensor("outp", (S, D), f16, kind="ExternalOutput")

    xp_ap, wqkv_ap, wout_ap, masks_ap, outp_ap = (
        xp.ap(), wqkv.ap(), wout.ap(), masks.ap(), outp.ap())

    with tile.TileContext(nc, trace_sim=CFG.get('TRACE', False)) as tc:
        with tc.tile_pool(name="const", bufs=1) as const, \
             tc.tile_pool(name="persist", bufs=1) as persist:

            ident = const.tile([P, P], f32, tag="ident")
            make_identity(nc, ident)
            ident16 = const.tile([P, P], f16, tag="ident16")
            nc.scalar.copy(ident16, ident)
            masks_sb = const.tile([P, 4, 512], f16, tag="masks")
            bq_sb = const.tile([P, HC], f32, tag="bq")
            bk_sb = const.tile([P, HC], f32, tag="bk")
            bvb_sb = const.tile([P, HC * Dh], f32, tag="bvb")

            # resident weights: per-(part, head) DMAs so the first projection
            # chains can start while the rest stream in; spread across queues
            w_sb = persist.tile([P, 3, HC, N_DC, P], f16, tag="w")
            for pi in range(3):
                for h in range(HC):
                    eng = nc.scalar if (pi * HC + h) % 2 == 0 else nc.gpsimd
                    eng.dma_start(w_sb[:, pi, h], wqkv_ap[:, pi, h])
            wo_sb = persist.tile([P, HC, D], f16, tag="wo")
            nc.gpsimd.dma_start(wo_sb[:], wout_ap)

            kT_sb = persist.tile([P, HC, S], f16, tag="kT")
            vaug_sb = persist.tile([P, HC, N_SC, VW], f16, tag="vaug")
            attnT_sb = persist.tile([P, HC, S], f16, tag="attnT")
            # ones column for softmax denominators
            nc.vector.memset(vaug_sb[:, :, :, Dh:Dh + 1], 1.0)

            with tc.tile_pool(name="xpool", bufs=CFG["XB"]) as xpool, \
                 tc.tile_pool(name="qpool", bufs=CFG["QB"]) as qpool, \
                 tc.tile_pool(name="vpool", bufs=CFG["VB"]) as vpool, \
                 tc.tile_pool(name="exppool", bufs=CFG["EXPB"]) as exppool, \
                 tc.tile_pool(name="spool", bufs=CFG["SPB"]) as spool, \
                 tc.tile_pool(name="osb", bufs=CFG["OSB"]) as osb, \
                 tc.tile_pool(name="bigps", bufs=CFG["BIGPS"], space="PSUM") as bigps, \
                 tc.tile_pool(name="pvps", bufs=CFG["PVB"], space="PSUM") as pvps, \
                 tc.tile_pool(name="trps", bufs=CFG["TRB"], space="PSUM") as trps:

                for sf in range(SPLIT):
                    # ---- Phase A: qkv projection for this 512-query chunk ----
                    xT_sb = xpool.tile([P, N_DC, SQ], f16, tag="xT")
                    nc.sync.dma_start(xT_sb[:, 0:N_DC // 2],
                                      xp_ap[:, sf, 0:N_DC // 2])
                    nc.sync.dma_start(xT_sb[:, N_DC // 2:],
                                      xp_ap[:, sf, N_DC // 2:])
                    if sf == 0:
                        for j in range(4):
                            nc.scalar.dma_start(masks_sb[:, j], masks_ap[j])
                        nc.gpsimd.dma_start(bq_sb[:], bq.ap())
                        nc.gpsimd.dma_start(bk_sb[:], bk.ap())
                        nc.gpsimd.dma_start(bvb_sb[:], bvb.ap())
                    qT_sb = qpool.tile([P, HC, SQ], f16, tag="qT")
                    vT_sb = vpool.tile([P, HC, SQ], f16, tag="vT")

                    for h in range(HC):
                        for pi, part in enumerate(("q", "k", "v")):
                            ps_t = bigps.tile([P, 512], f32, tag="bigps",
                                              name=f"proj{sf}_{h}_{part}")
                            for dc in range(N_DC):
                                nc.tensor.matmul(
                                    ps_t,
                                    w_sb[:, pi, h, dc],
                                    xT_sb[:, dc],
                                    start=(dc == 0), stop=(dc == N_DC - 1))
                            if part == "q":
                                nc.vector.tensor_scalar_add(
                                    qT_sb[:, h], ps_t, bq_sb[:, h:h + 1])
                            elif part == "k":
                                nc.vector.tensor_scalar_add(
                                    kT_sb[:, h, sf * SQ:(sf + 1) * SQ],
                                    ps_t, bk_sb[:, h:h + 1])
                            else:
                                nc.vector.tensor_copy(vT_sb[:, h], ps_t)

                    # v: transpose to natural layout, add bias (f16)
                    for h in range(HC):
                        for scl in range(SQ // P):
                            sc = sf * (SQ // P) + scl
                            tp = trps.tile([P, P], f16, tag="tr")
                            nc.tensor.transpose(
                                tp, vT_sb[:, h, scl * P:(scl + 1) * P], ident16)
                            nc.vector.tensor_add(
                                vaug_sb[:, h, sc, 0:Dh], tp,
                                bvb_sb[:, h * Dh:(h + 1) * Dh])

                    # ---- Phase B: attention for the 512 queries of this chunk ----
                    q0 = sf * SQ
                    nk = (q0 + 512) // P        # key chunks needed (causal)
                    for h in range(HC):
                        exps = []
                        for kc in range(nk):
                            j = kc - (nk - 4)
                            # columns < j*128 of this chunk are fully masked
                            off = 0 if j < 1 else j * P
                            qk = bigps.tile([P, 512], f32, tag="bigps",
                                            name=f"qk{sf}_{h}_{kc}")
                            nc.tensor.matmul(
                                qk[:, 0:512 - off],
                                kT_sb[:, h, kc * P:(kc + 1) * P],
                                qT_sb[:, h, off:512],
                                start=True, stop=True)
                            ex = exppool.tile([P, 512], f16, tag="exp")
                            nc.scalar.activation(
                                ex[:, off:512], qk[:, 0:512 - off],
                                mybir.ActivationFunctionType.Exp,
                                scale=SCALE)
                            if j >= 0:
                                nc.vector.tensor_mul(
                                    ex[:, off:512], ex[:, off:512],
                                    masks_sb[:, j, off:512])
                            exps.append(ex)
                        for sub in range(4):
                            nkq = sf * 4 + sub + 1
                            ps = pvps.tile([P, VW], f32, tag="pv")
                            for kc in range(nkq):
                                nc.tensor.matmul(
                                    ps[:, 0:Dh + 1],
                                    exps[kc][:, sub * P:(sub + 1) * P],
                                    vaug_sb[:, h, kc, 0:Dh + 1],
                                    start=(kc == 0),
                                    stop=(kc == nkq - 1))
                            rc = spool.tile([P, 1], f32, tag="rc")
                            nc.vector.reciprocal(rc, ps[:, Dh:Dh + 1])
                            at = spool.tile([P, P], f16, tag="at")
                            nc.vector.tensor_mul(
                                at, ps[:, 0:Dh], rc.to_broadcast((P, P)))
                            tp = trps.tile([P, P], f16, tag="tr")
                            nc.tensor.transpose(tp, at, ident16)
                            nc.vector.tensor_copy(
                                attnT_sb[:, h,
                                         q0 + sub * P:q0 + (sub + 1) * P],
                                tp)

                    # ---- Phase C: partial out projection for this chunk ----
                    for ssl in range(SQ // P):
                        ss = sf * (SQ // P) + ssl
                        ot = osb.tile([P, D], f16, tag="ot")
                        for n in range(D // 512):
                            ps_o = bigps.tile([P, 512], f32, tag="bigps",
                                              name=f"ops{ss}_{n}")
                            for hc in range(HC):
                                nc.tensor.matmul(
                                    ps_o,
                                    attnT_sb[:, hc, ss * P:(ss + 1) * P],
                                    wo_sb[:, hc, n * 512:(n + 1) * 512],
                                    start=(hc == 0),
                                    stop=(hc == HC - 1))
                            # balance PSUM->SBUF copies across DVE and ACT
                            if n % 4 != 3:
                                nc.vector.tensor_copy(
                                    ot[:, n * 512:(n + 1) * 512], ps_o)
                            else:
                                nc.scalar.copy(
                                    ot[:, n * 512:(n + 1) * 512], ps_o)
                        eng = nc.sync if ssl % 2 == 0 else nc.gpsimd
                        eng.dma_start(outp_ap[ss * P:(ss + 1) * P], ot)

    nc.compile()
    return nc


def shard_inputs(x, Wqkv, bqkv, Wout):
    """Build the 8 per-core input maps."""
    mask = np.zeros((4, P, 512), np.float16)
    kk = np.arange(P)[:, None]
    qq = np.arange(512)[None, :]
    for j in range(4):
        mask[j] = (qq >= kk + P * j).astype(np.float16)

    # x packed [p, sf, dc, sq] per batch (f16): each per-sf slice is 16KB
    # contiguous per partition
    xps = [np.ascontiguousarray(
        x[b].T.astype(np.float16).reshape(N_DC, P, SPLIT, SQ)
        .transpose(1, 2, 0, 3)) for b in range(B)]
    # Wqkv [D, 3*H*Dh] -> [dc, p, 3, H, dh] (f16)
    W_all = Wqkv.astype(np.float16).reshape(N_DC, P, 3, H, Dh)
    wout_f16 = Wout.astype(np.float16)
    per_hg = {}
    for hg in range(4):
        h0 = hg * HC
        c0 = h0 * Dh
        cw = HC * Dh
        # [p, part, h', dc, dh]: each (part, h') slice is 4KB/partition
        per_hg[hg] = dict(
            wqkv=np.ascontiguousarray(
                W_all[:, :, :, h0:h0 + HC]
                .transpose(1, 2, 3, 0, 4)),
            wout=np.ascontiguousarray(
                wout_f16[c0:c0 + cw].reshape(HC, P, D).transpose(1, 0, 2)),
            bq=np.ascontiguousarray(
                bqkv[c0:c0 + cw].reshape(HC, P).T).astype(np.float32),
            bk=np.ascontiguousarray(
                bqkv[H * Dh + c0:H * Dh + c0 + cw]
                .reshape(HC, P).T).astype(np.float32),
            bvb=np.ascontiguousarray(np.broadcast_to(
                bqkv[2 * H * Dh + c0:2 * H * Dh + c0 + cw]
                .astype(np.float32)[None, :], (P, cw))),
        )
    in_maps = []
    for c in range(8):
        b, hg = divmod(c, 4)
        g = per_hg[hg]
        in_maps.append({
            "xp": xps[b], "wqkv": g["wqkv"],
            "wout": g["wout"], "bq": g["bq"], "bk": g["bk"], "bvb": g["bvb"],
            "masks": mask,
        })
    return in_maps


def _prepare():
    """Compile the bass program once and build a cached sharded jit."""
    import jax
    from jax.sharding import Mesh, PartitionSpec
    from jax.experimental.shard_map import shard_map
    from concourse import bass2jax
    from concourse import mybir as mb

    nc = build_program()
    bass2jax.install_neuronx_cc_hook()
    partition_name = (nc.partition_id_tensor.name
                      if nc.partition_id_tensor else None)
    in_names, out_names, out_avals, zero_outs = [], [], [], []
    for alloc in nc.m.functions[0].allocations:
        if not isinstance(alloc, mb.MemoryLocationSet):
            continue
        name = alloc.memorylocations[0].name
        if alloc.kind == "ExternalInput":
            if name != partition_name:
                in_names.append(name)
        elif alloc.kind == "ExternalOutput":
            shape = tuple(alloc.tensor_shape)
            dtype = mb.dt.np(alloc.dtype)
            out_names.append(name)
            out_avals.append(jax.core.ShapedArray(shape, dtype))
            zero_outs.append(np.zeros(shape, dtype))
    n_params, n_outs = len(in_names), len(out_names)
    all_in_names = (in_names + out_names
                    + ([partition_name] if partition_name else []))

    def _body(*args):
        operands = list(args)
        if partition_name is not None:
            operands.append(bass2jax.partition_id_tensor())
        outs = bass2jax._bass_exec_p.bind(
            *operands,
            out_avals=tuple(out_avals),
            in_names=tuple(all_in_names),
            out_names=tuple(out_names),
            lowering_input_output_aliases=(),
            sim_require_finite=True,
            sim_require_nnan=True,
            nc=nc,
        )
        return tuple(outs)

    n_cores = 8
    devices = jax.devices()[:n_cores]
    mesh = Mesh(np.asarray(devices), ("core",))
    sharded = jax.jit(
        shard_map(_body, mesh=mesh,
                  in_specs=(PartitionSpec("core"),) * (n_params + n_outs),
                  out_specs=(PartitionSpec("core"),) * n_outs,
                  check_rep=False),
        donate_argnums=tuple(range(n_params, n_params + n_outs)),
        keep_unused=True,
    )
    return dict(nc=nc, sharded=sharded, in_names=in_names,
                zero_outs=zero_outs, n_cores=n_cores)


def kernel(x, Wqkv, bqkv, Wout, bout):
    import jax

    x = np.asarray(x, dtype=np.float32)
    Wqkv = np.asarray(Wqkv, dtype=np.float32)
    bqkv = np.asarray(bqkv, dtype=np.float32)
    Wout = np.asarray(Wout, dtype=np.float32)
    bout = np.asarray(bout, dtype=np.float32)

    if "ctx" not in _COMPILED:
        _COMPILED["ctx"] = _prepare()
        _COMPILED["nc"] = _COMPILED["ctx"]["nc"]
    ctx = _COMPILED["ctx"]
    n_cores = ctx["n_cores"]

    in_maps = shard_inputs(x, Wqkv, bqkv, Wout)
    per_core = [[np.asarray(m[nm]) for nm in ctx["in_names"]]
                for m in in_maps]
    concat_in = [np.concatenate([per_core[c][i] for c in range(n_cores)],
                                axis=0)
                 for i in range(len(ctx["in_names"]))]
    zs = [np.zeros((n_cores * z.shape[0], *z.shape[1:]), z.dtype)
          for z in ctx["zero_outs"]]
    outs = ctx["sharded"](*concat_in, *zs)
    jax.block_until_ready(outs)
    outp = np.asarray(outs[0])  # [8*S, D] f16, core-major

    out = np.empty((B, S, D), np.float32)
    for b in range(B):
        acc = outp[4 * b * S:(4 * b + 1) * S].astype(np.float32)
        for c in range(4 * b + 1, 4 * b + 4):
            acc += outp[c * S:(c + 1) * S].astype(np.float32)
        out[b] = acc + bout[None, :]
    return out



# revision 2
# speedup vs baseline: 4.2765x; 4.2765x over previous
"""Chunkwise causal attention (full causal MHA + QKV/out projections) on 8 trn2 cores.

v3: PE transposes replaced by XBAR DMA transposes (dma_start_transpose),
consumption-ordered startup DMAs (w blocks h-major, masks/wout deferred),
PSUM rebalance (6 accumulation banks), per-sub attnT transposes for the
last head so the out-projection isn't gated on DMA-transpose latency.

Sharding: data-parallel over batch (B=2) x tensor-parallel over heads (16 -> 4 per
core). Host sums the 4 partial out-projections per batch and adds bout.

Self-contained: hardcodes all shapes from the problem spec.
"""

import numpy as np

import concourse.bass as bass
import concourse.mybir as mybir
import concourse.tile as tile
from concourse import bacc

# Problem shapes
B, S, D = 2, 2048, 2048
H, Dh = 16, 128
HC = 4                      # heads per core
P = 128
SPLIT = 4
SQ = S // SPLIT             # 512 queries per outer phase
N_DC = D // P               # 16 contraction chunks for projections
N_SC = S // P               # 16 key chunks
SCALE = 1.0 / float(np.sqrt(Dh))
VW = 132                    # vaug row width (129 used: 128 dh + 1 ones col)

f32 = mybir.dt.float32
f16 = mybir.dt.float16

_COMPILED = {}

CFG = dict(XB=2, QB=2, VB=2, EXPB=20, SPB=4, OSB=3, BIGPS=6, PVB=2)


def build_program():
    nc = bacc.Bacc("TRN2", target_bir_lowering=False, debug=False)

    xp = nc.dram_tensor("xp", (P, SPLIT, N_DC, SQ), f16, kind="ExternalInput")
    wqkv = nc.dram_tensor("wqkv", (P, 3, HC, N_DC, P), f16, kind="ExternalInput")
    wout = nc.dram_tensor("wout", (P, HC, D), f16, kind="ExternalInput")
    bq = nc.dram_tensor("bq", (P, HC), f32, kind="ExternalInput")
    bk = nc.dram_tensor("bk", (P, HC), f32, kind="ExternalInput")
    bv = nc.dram_tensor("bv", (P, HC), f32, kind="ExternalInput")
    masks = nc.dram_tensor("masks", (4, P, 512), f16, kind="ExternalInput")
    outp = nc.dram_tensor("outp", (S, D), f16, kind="ExternalOutput")

    xp_ap, wqkv_ap, wout_ap, masks_ap, outp_ap = (
        xp.ap(), wqkv.ap(), wout.ap(), masks.ap(), outp.ap())

    with tile.TileContext(nc, trace_sim=CFG.get('TRACE', False)) as tc:
        with tc.tile_pool(name="const", bufs=1) as const, \
             tc.tile_pool(name="persist", bufs=1) as persist:

            masks_sb = const.tile([P, 4, 512], f16, tag="masks")
            bq_sb = const.tile([P, HC], f32, tag="bq")
            bk_sb = const.tile([P, HC], f32, tag="bk")
            bv_sb = const.tile([P, HC], f32, tag="bv")

            w_sb = persist.tile([P, 3, HC, N_DC, P], f16, tag="w")
            wo_sb = persist.tile([P, HC, D], f16, tag="wo")
            kT_sb = persist.tile([P, HC, S], f16, tag="kT")
            vaug_sb = persist.tile([P, HC, N_SC, VW], f16, tag="vaug")
            attnT_sb = persist.tile([P, HC, N_SC, P], f16, tag="attnT")
            # ones column for softmax denominators
            nc.vector.memset(vaug_sb[:, :, :, Dh:Dh + 1], 1.0)

            with tc.tile_pool(name="xpool", bufs=CFG["XB"]) as xpool, \
                 tc.tile_pool(name="qpool", bufs=CFG["QB"]) as qpool, \
                 tc.tile_pool(name="vpool", bufs=CFG["VB"]) as vpool, \
                 tc.tile_pool(name="exppool", bufs=CFG["EXPB"]) as exppool, \
                 tc.tile_pool(name="spool", bufs=CFG["SPB"]) as spool, \
                 tc.tile_pool(name="osb", bufs=CFG["OSB"]) as osb, \
                 tc.tile_pool(name="bigps", bufs=CFG["BIGPS"], space="PSUM") as bigps, \
                 tc.tile_pool(name="pvps", bufs=CFG["PVB"], space="PSUM") as pvps:

                # ---- startup DMAs, ordered by first use ----
                # x chunk for sf=0 in dc-quarters so the first chain can
                # start after ~1/4 of the transfer
                xT0 = xpool.tile([P, N_DC, SQ], f16, tag="xT")
                for qtr in range(4):
                    nc.sync.dma_start(xT0[:, 4 * qtr:4 * qtr + 4],
                                      xp_ap[:, 0, 4 * qtr:4 * qtr + 4])
                nc.sync.dma_start(bq_sb[:], bq.ap())
                nc.sync.dma_start(bk_sb[:], bk.ap())
                nc.sync.dma_start(bv_sb[:], bv.ap())
                # w blocks in consumption order (h-major), alternating queues
                for c in range(3 * HC):
                    h, pi = divmod(c, 3)
                    eng = nc.scalar if c % 2 == 0 else nc.gpsimd
                    eng.dma_start(w_sb[:, pi, h], wqkv_ap[:, pi, h])
                # later-needed constants after the w stream
                for j in range(4):
                    nc.scalar.dma_start(masks_sb[:, j], masks_ap[j])
                for hc in range(HC):
                    eng = nc.scalar if hc % 2 == 0 else nc.gpsimd
                    eng.dma_start(wo_sb[:, hc], wout_ap[:, hc])

                for sf in range(SPLIT):
                    # ---- Phase A: qkv projection for this 512-query chunk ----
                    if sf == 0:
                        xT_sb = xT0
                    else:
                        xT_sb = xT_next  # noqa: F821 (prefetched below)
                    qT_sb = qpool.tile([P, HC, SQ], f16, tag="qT")
                    vT_sb = vpool.tile([P, HC, SQ], f16, tag="vT")

                    for h in range(HC):
                        for pi, part in enumerate(("q", "k", "v")):
                            ps_t = bigps.tile([P, 512], f32, tag="bigps",
                                              name=f"proj{sf}_{h}_{part}")
                            for dc in range(N_DC):
                                nc.tensor.matmul(
                                    ps_t,
                                    w_sb[:, pi, h, dc],
                                    xT_sb[:, dc],
                                    start=(dc == 0), stop=(dc == N_DC - 1))
                            if part == "q":
                                nc.vector.tensor_scalar_add(
                                    qT_sb[:, h], ps_t, bq_sb[:, h:h + 1])
                            elif part == "k":
                                nc.vector.tensor_scalar_add(
                                    kT_sb[:, h, sf * SQ:(sf + 1) * SQ],
                                    ps_t, bk_sb[:, h:h + 1])
                            else:
                                nc.vector.tensor_scalar_add(
                                    vT_sb[:, h], ps_t, bv_sb[:, h:h + 1])
                                # v -> natural [key, dh] layout via XBAR DMA
                                nc.sync.dma_start_transpose(
                                    vaug_sb[:, h, sf * 4:(sf + 1) * 4, 0:Dh],
                                    vT_sb[:, h])

                    # prefetch next x chunk
                    if sf < SPLIT - 1:
                        xT_next = xpool.tile([P, N_DC, SQ], f16, tag="xT")
                        for qtr in range(4):
                            nc.sync.dma_start(
                                xT_next[:, 4 * qtr:4 * qtr + 4],
                                xp_ap[:, sf + 1, 4 * qtr:4 * qtr + 4])

                    # ---- Phase B: attention for the 512 queries of this chunk ----
                    q0 = sf * SQ
                    nk = (q0 + 512) // P        # key chunks needed (causal)
                    for h in range(HC):
                        exps = []
                        for kc in range(nk):
                            j = kc - (nk - 4)
                            # columns < j*128 of this chunk are fully masked
                            off = 0 if j < 1 else j * P
                            qk = bigps.tile([P, 512], f32, tag="bigps",
                                            name=f"qk{sf}_{h}_{kc}")
                            nc.tensor.matmul(
                                qk[:, 0:512 - off],
                                kT_sb[:, h, kc * P:(kc + 1) * P],
                                qT_sb[:, h, off:512],
                                start=True, stop=True)
                            ex = exppool.tile([P, 512], f16, tag="exp")
                            nc.scalar.activation(
                                ex[:, off:512], qk[:, 0:512 - off],
                                mybir.ActivationFunctionType.Exp,
                                scale=SCALE)
                            if j >= 0:
                                nc.vector.tensor_mul(
                                    ex[:, off:512], ex[:, off:512],
                                    masks_sb[:, j, off:512])
                            exps.append(ex)
                        at_all = spool.tile([P, 4, P], f16, tag="at")
                        for sub in range(4):
                            nkq = sf * 4 + sub + 1
                            ps = pvps.tile([P, VW], f32, tag="pv")
                            for kc in range(nkq):
                                nc.tensor.matmul(
                                    ps[:, 0:Dh + 1],
                                    exps[kc][:, sub * P:(sub + 1) * P],
                                    vaug_sb[:, h, kc, 0:Dh + 1],
                                    start=(kc == 0),
                                    stop=(kc == nkq - 1))
                            rc = spool.tile([P, 1], f32, tag="rc")
                            nc.vector.reciprocal(rc, ps[:, Dh:Dh + 1])
                            nc.vector.tensor_mul(
                                at_all[:, sub], ps[:, 0:Dh],
                                rc.to_broadcast((P, P)))
                            if h == HC - 1:
                                # last head: per-sub transpose so Phase C
                                # isn't gated on the full-tile DMA
                                nc.sync.dma_start_transpose(
                                    attnT_sb[:, h, sf * 4 + sub, :],
                                    at_all[:, sub, :])
                        if h < HC - 1:
                            nc.sync.dma_start_transpose(
                                attnT_sb[:, h, sf * 4:(sf + 1) * 4, :],
                                at_all[:])

                    # ---- Phase C: partial out projection for this chunk ----
                    for ssl in range(SQ // P):
                        ss = sf * (SQ // P) + ssl
                        ot = osb.tile([P, D], f16, tag="ot")
                        for n in range(D // 512):
                            ps_o = bigps.tile([P, 512], f32, tag="bigps",
                                              name=f"ops{ss}_{n}")
                            for hc in range(HC):
                                nc.tensor.matmul(
                                    ps_o,
                                    attnT_sb[:, hc, ss, :],
                                    wo_sb[:, hc, n * 512:(n + 1) * 512],
                                    start=(hc == 0),
                                    stop=(hc == HC - 1))
                            # balance PSUM->SBUF copies across DVE and ACT
                            if n % 4 != 3:
                                nc.vector.tensor_copy(
                                    ot[:, n * 512:(n + 1) * 512], ps_o)
                            else:
                                nc.scalar.copy(
                                    ot[:, n * 512:(n + 1) * 512], ps_o)
                        nc.gpsimd.dma_start(outp_ap[ss * P:(ss + 1) * P], ot)

    nc.compile()
    return nc


def shard_inputs(x, Wqkv, bqkv, Wout):
    """Build the 8 per-core input maps."""
    mask = np.zeros((4, P, 512), np.float16)
    kk = np.arange(P)[:, None]
    qq = np.arange(512)[None, :]
    for j in range(4):
        mask[j] = (qq >= kk + P * j).astype(np.float16)

    # x packed [p, sf, dc, sq] per batch (f16): each per-sf slice is 16KB
    # contiguous per partition
    xps = [np.ascontiguousarray(
        x[b].T.astype(np.float16).reshape(N_DC, P, SPLIT, SQ)
        .transpose(1, 2, 0, 3)) for b in range(B)]
    # Wqkv [D, 3*H*Dh] -> [dc, p, 3, H, dh] (f16)
    W_all = Wqkv.astype(np.float16).reshape(N_DC, P, 3, H, Dh)
    wout_f16 = Wout.astype(np.float16)
    per_hg = {}
    for hg in range(4):
        h0 = hg * HC
        c0 = h0 * Dh
        cw = HC * Dh
        # [p, part, h', dc, dh]: each (part, h') slice is 4KB/partition
        per_hg[hg] = dict(
            wqkv=np.ascontiguousarray(
                W_all[:, :, :, h0:h0 + HC]
                .transpose(1, 2, 3, 0, 4)),
            wout=np.ascontiguousarray(
                wout_f16[c0:c0 + cw].reshape(HC, P, D).transpose(1, 0, 2)),
            bq=np.ascontiguousarray(
                bqkv[c0:c0 + cw].reshape(HC, P).T).astype(np.float32),
            bk=np.ascontiguousarray(
                bqkv[H * Dh + c0:H * Dh + c0 + cw]
                .reshape(HC, P).T).astype(np.float32),
            bv=np.ascontiguousarray(
                bqkv[2 * H * Dh + c0:2 * H * Dh + c0 + cw]
                .reshape(HC, P).T).astype(np.float32),
        )
    in_maps = []
    for c in range(8):
        b, hg = divmod(c, 4)
        g = per_hg[hg]
        in_maps.append({
            "xp": xps[b], "wqkv": g["wqkv"],
            "wout": g["wout"], "bq": g["bq"], "bk": g["bk"], "bv": g["bv"],
            "masks": mask,
        })
    return in_maps


def _prepare():
    """Compile the bass program once and build a cached sharded jit."""
    import jax
    from jax.sharding import Mesh, PartitionSpec
    from jax.experimental.shard_map import shard_map
    from concourse import bass2jax
    from concourse import mybir as mb

    nc = build_program()
    bass2jax.install_neuronx_cc_hook()
    partition_name = (nc.partition_id_tensor.name
                      if nc.partition_id_tensor else None)
    in_names, out_names, out_avals, zero_outs = [], [], [], []
    for alloc in nc.m.functions[0].allocations:
        if not isinstance(alloc, mb.MemoryLocationSet):
            continue
        name = alloc.memorylocations[0].name
        if alloc.kind == "ExternalInput":
            if name != partition_name:
                in_names.append(name)
        elif alloc.kind == "ExternalOutput":
            shape = tuple(alloc.tensor_shape)
            dtype = mb.dt.np(alloc.dtype)
            out_names.append(name)
            out_avals.append(jax.core.ShapedArray(shape, dtype))
            zero_outs.append(np.zeros(shape, dtype))
    n_params, n_outs = len(in_names), len(out_names)
    all_in_names = (in_names + out_names
                    + ([partition_name] if partition_name else []))

    def _body(*args):
        operands = list(args)
        if partition_name is not None:
            operands.append(bass2jax.partition_id_tensor())
        outs = bass2jax._bass_exec_p.bind(
            *operands,
            out_avals=tuple(out_avals),
            in_names=tuple(all_in_names),
            out_names=tuple(out_names),
            lowering_input_output_aliases=(),
            sim_require_finite=True,
            sim_require_nnan=True,
            nc=nc,
        )
        return tuple(outs)

    n_cores = 8
    devices = jax.devices()[:n_cores]
    mesh = Mesh(np.asarray(devices), ("core",))
    sharded = jax.jit(
        shard_map(_body, mesh=mesh,
                  in_specs=(PartitionSpec("core"),) * (n_params + n_outs),
                  out_specs=(PartitionSpec("core"),) * n_outs,
                  check_rep=False),
        donate_argnums=tuple(range(n_params, n_params + n_outs)),
        keep_unused=True,
    )
    return dict(nc=nc, sharded=sharded, in_names=in_names,
                zero_outs=zero_outs, n_cores=n_cores)


def kernel(x, Wqkv, bqkv, Wout, bout):
    import jax

    x = np.asarray(x, dtype=np.float32)
    Wqkv = np.asarray(Wqkv, dtype=np.float32)
    bqkv = np.asarray(bqkv, dtype=np.float32)
    Wout = np.asarray(Wout, dtype=np.float32)
    bout = np.asarray(bout, dtype=np.float32)

    if "ctx" not in _COMPILED:
        _COMPILED["ctx"] = _prepare()
        _COMPILED["nc"] = _COMPILED["ctx"]["nc"]
    ctx = _COMPILED["ctx"]
    n_cores = ctx["n_cores"]

    in_maps = shard_inputs(x, Wqkv, bqkv, Wout)
    per_core = [[np.asarray(m[nm]) for nm in ctx["in_names"]]
                for m in in_maps]
    concat_in = [np.concatenate([per_core[c][i] for c in range(n_cores)],
                                axis=0)
                 for i in range(len(ctx["in_names"]))]
    zs = [np.zeros((n_cores * z.shape[0], *z.shape[1:]), z.dtype)
          for z in ctx["zero_outs"]]
    outs = ctx["sharded"](*concat_in, *zs)
    jax.block_until_ready(outs)
    outp = np.asarray(outs[0])  # [8*S, D] f16, core-major

    out = np.empty((B, S, D), np.float32)
    for b in range(B):
        acc = outp[4 * b * S:(4 * b + 1) * S].astype(np.float32)
        for c in range(4 * b + 1, 4 * b + 4):
            acc += outp[c * S:(c + 1) * S].astype(np.float32)
        out[b] = acc + bout[None, :]
    return out


# revision 9
# speedup vs baseline: 4.4316x; 1.0363x over previous
"""Chunkwise causal attention (full causal MHA + QKV/out projections) on 8 trn2 cores.

v6: all-f16 datapath (f32 PSUM accumulation), PE transposes (XBAR DMA
transpose corrupts on HW despite passing CoreSim), consumption-ordered
startup DMAs (w blocks h-major on the SWDGE rings, masks/wout deferred,
sf0 x in dc-quarters), x prefetch emitted right after Phase A, v-bias
folded into the PSUM drain.

Sharding: data-parallel over batch (B=2) x tensor-parallel over heads (16 -> 4 per
core). Host sums the 4 partial out-projections per batch and adds bout.

Self-contained: hardcodes all shapes from the problem spec.
"""

import numpy as np

import concourse.bass as bass
import concourse.mybir as mybir
import concourse.tile as tile
from concourse import bacc
from concourse.masks import make_identity

# Problem shapes
B, S, D = 2, 2048, 2048
H, Dh = 16, 128
HC = 4                      # heads per core
P = 128
SPLIT = 4
SQ = S // SPLIT             # 512 queries per outer phase
N_DC = D // P               # 16 contraction chunks for projections
N_SC = S // P               # 16 key chunks
SCALE = 1.0 / float(np.sqrt(Dh))
VW = 132                    # vaug row width (129 used: 128 dh + 1 ones col)

f32 = mybir.dt.float32
f16 = mybir.dt.float16

_COMPILED = {}

CFG = dict(XB=2, QB=2, VB=2, EXPB=20, SPB=4, OSB=3, BIGPS=5, PVB=2, TRB=1)


def build_program():
    nc = bacc.Bacc("TRN2", target_bir_lowering=False, debug=False)

    xp = nc.dram_tensor("xp", (P, SPLIT, N_DC, SQ), f16, kind="ExternalInput")
    wqkv = nc.dram_tensor("wqkv", (P, 3, HC, N_DC, P), f16, kind="ExternalInput")
    wout = nc.dram_tensor("wout", (P, HC, D), f16, kind="ExternalInput")
    bq = nc.dram_tensor("bq", (P, HC), f32, kind="ExternalInput")
    bk = nc.dram_tensor("bk", (P, HC), f32, kind="ExternalInput")
    bv = nc.dram_tensor("bv", (P, HC), f32, kind="ExternalInput")
    masks = nc.dram_tensor("masks", (4, P, 512), f16, kind="ExternalInput")
    outp = nc.dram_tensor("outp", (S, D), f16, kind="ExternalOutput")

    xp_ap, wqkv_ap, wout_ap, masks_ap, outp_ap = (
        xp.ap(), wqkv.ap(), wout.ap(), masks.ap(), outp.ap())

    with tile.TileContext(nc, trace_sim=CFG.get('TRACE', False)) as tc:
        with tc.tile_pool(name="const", bufs=1) as const, \
             tc.tile_pool(name="persist", bufs=1) as persist:

            ident = const.tile([P, P], f32, tag="ident")
            make_identity(nc, ident)
            ident16 = const.tile([P, P], f16, tag="ident16")
            nc.scalar.copy(ident16, ident)
            masks_sb = const.tile([P, 4, 512], f16, tag="masks")
            bq_sb = const.tile([P, HC], f32, tag="bq")
            bk_sb = const.tile([P, HC], f32, tag="bk")
            bv_sb = const.tile([P, HC], f32, tag="bv")

            w_sb = persist.tile([P, 3, HC, N_DC, P], f16, tag="w")
            wo_sb = persist.tile([P, HC, D], f16, tag="wo")
            kT_sb = persist.tile([P, HC, S], f16, tag="kT")
            vaug_sb = persist.tile([P, HC, N_SC, VW], f16, tag="vaug")
            attnT_sb = persist.tile([P, HC, N_SC, P], f16, tag="attnT")
            # ones column for softmax denominators
            nc.vector.memset(vaug_sb[:, :, :, Dh:Dh + 1], 1.0)

            with tc.tile_pool(name="xpool", bufs=CFG["XB"]) as xpool, \
                 tc.tile_pool(name="qpool", bufs=CFG["QB"]) as qpool, \
                 tc.tile_pool(name="vpool", bufs=CFG["VB"]) as vpool, \
                 tc.tile_pool(name="exppool", bufs=CFG["EXPB"]) as exppool, \
                 tc.tile_pool(name="spool", bufs=CFG["SPB"]) as spool, \
                 tc.tile_pool(name="osb", bufs=CFG["OSB"]) as osb, \
                 tc.tile_pool(name="bigps", bufs=CFG["BIGPS"], space="PSUM") as bigps, \
                 tc.tile_pool(name="pvps", bufs=CFG["PVB"], space="PSUM") as pvps, \
                 tc.tile_pool(name="trps", bufs=CFG["TRB"], space="PSUM") as trps:

                # ---- startup DMAs, ordered by first use ----
                # sf0 x + tiny biases on the SP/HWDGE rings; all bulk
                # (w/masks/wout/prefetch/output) on the Pool SWDGE rings.
                # sf0 x in 2-dc slivers: the first chain's dc0 gate is a
                # 256KB transfer, not 512KB, under fair-share DMA service
                xT0 = xpool.tile([P, N_DC, SQ], f16, tag="xT")
                for qtr in range(8):
                    nc.sync.dma_start(xT0[:, 2 * qtr:2 * qtr + 2],
                                      xp_ap[:, 0, 2 * qtr:2 * qtr + 2])
                nc.sync.dma_start(bq_sb[:], bq.ap())
                nc.sync.dma_start(bk_sb[:], bk.ap())
                nc.sync.dma_start(bv_sb[:], bv.ap())
                # w blocks in consumption order (h-major); h0's blocks in
                # dc-halves so the first chains aren't gated on full blocks
                for c in range(3 * HC):
                    h, pi = divmod(c, 3)
                    if h == 0:
                        nc.gpsimd.dma_start(w_sb[:, pi, h, 0:N_DC // 2],
                                            wqkv_ap[:, pi, h, 0:N_DC // 2])
                        nc.gpsimd.dma_start(w_sb[:, pi, h, N_DC // 2:],
                                            wqkv_ap[:, pi, h, N_DC // 2:])
                    else:
                        nc.gpsimd.dma_start(w_sb[:, pi, h], wqkv_ap[:, pi, h])
                for j in range(4):
                    nc.gpsimd.dma_start(masks_sb[:, j], masks_ap[j])
                for hc in range(HC):
                    nc.gpsimd.dma_start(wo_sb[:, hc], wout_ap[:, hc])

                for sf in range(SPLIT):
                    # ---- Phase A: qkv projection for this 512-query chunk ----
                    if sf == 0:
                        xT_sb = xT0
                    else:
                        xT_sb = xT_next  # noqa: F821 (prefetched below)
                    qT_sb = qpool.tile([P, HC, SQ], f16, tag="qT")
                    vT_sb = vpool.tile([P, HC, SQ], f16, tag="vT")

                    for h in range(HC):
                        for pi, part in enumerate(("q", "k", "v")):
                            ps_t = bigps.tile([P, 512], f32, tag="bigps",
                                              name=f"proj{sf}_{h}_{part}")
                            for dc in range(N_DC):
                                nc.tensor.matmul(
                                    ps_t,
                                    w_sb[:, pi, h, dc],
                                    xT_sb[:, dc],
                                    start=(dc == 0), stop=(dc == N_DC - 1))
                            if part == "q":
                                nc.vector.tensor_scalar_add(
                                    qT_sb[:, h], ps_t, bq_sb[:, h:h + 1])
                            elif part == "k":
                                nc.vector.tensor_scalar_add(
                                    kT_sb[:, h, sf * SQ:(sf + 1) * SQ],
                                    ps_t, bk_sb[:, h:h + 1])
                            else:
                                # v bias folded into the PSUM drain
                                nc.vector.tensor_scalar_add(
                                    vT_sb[:, h], ps_t, bv_sb[:, h:h + 1])

                    # v: transpose to natural [key, dh] layout on PE
                    for h in range(HC):
                        for scl in range(SQ // P):
                            sc = sf * (SQ // P) + scl
                            tp = trps.tile([P, P], f16, tag="tr")
                            nc.tensor.transpose(
                                tp, vT_sb[:, h, scl * P:(scl + 1) * P], ident16)
                            nc.vector.tensor_copy(
                                vaug_sb[:, h, sc, 0:Dh], tp)

                    # prefetch next x chunk on the SWDGE rings
                    if sf < SPLIT - 1:
                        xT_next = xpool.tile([P, N_DC, SQ], f16, tag="xT")
                        for qtr in range(4):
                            nc.gpsimd.dma_start(
                                xT_next[:, 4 * qtr:4 * qtr + 4],
                                xp_ap[:, sf + 1, 4 * qtr:4 * qtr + 4])

                    # ---- Phase B: attention for the 512 queries of this chunk ----
                    q0 = sf * SQ
                    nk = (q0 + 512) // P        # key chunks needed (causal)
                    for h in range(HC):
                        exps = []
                        for kc in range(nk):
                            j = kc - (nk - 4)
                            # columns < j*128 of this chunk are fully masked
                            off = 0 if j < 1 else j * P
                            qk = bigps.tile([P, 512], f32, tag="bigps",
                                            name=f"qk{sf}_{h}_{kc}")
                            nc.tensor.matmul(
                                qk[:, 0:512 - off],
                                kT_sb[:, h, kc * P:(kc + 1) * P],
                                qT_sb[:, h, off:512],
                                start=True, stop=True)
                            ex = exppool.tile([P, 512], f16, tag="exp")
                            nc.scalar.activation(
                                ex[:, off:512], qk[:, 0:512 - off],
                                mybir.ActivationFunctionType.Exp,
                                scale=SCALE)
                            if j >= 0:
                                nc.vector.tensor_mul(
                                    ex[:, off:512], ex[:, off:512],
                                    masks_sb[:, j, off:512])
                            exps.append(ex)
                        for sub in range(4):
                            nkq = sf * 4 + sub + 1
                            ps = pvps.tile([P, VW], f32, tag="pv")
                            for kc in range(nkq):
                                nc.tensor.matmul(
                                    ps[:, 0:Dh + 1],
                                    exps[kc][:, sub * P:(sub + 1) * P],
                                    vaug_sb[:, h, kc, 0:Dh + 1],
                                    start=(kc == 0),
                                    stop=(kc == nkq - 1))
                            rc = spool.tile([P, 1], f32, tag="rc")
                            nc.vector.reciprocal(rc, ps[:, Dh:Dh + 1])
                            at = spool.tile([P, P], f16, tag="at")
                            nc.vector.tensor_mul(
                                at, ps[:, 0:Dh], rc.to_broadcast((P, P)))
                            tp = trps.tile([P, P], f16, tag="tr")
                            nc.tensor.transpose(tp, at, ident16)
                            nc.vector.tensor_copy(
                                attnT_sb[:, h, sf * 4 + sub, :], tp)

                    # ---- Phase C: partial out projection for this chunk ----
                    for ssl in range(SQ // P):
                        ss = sf * (SQ // P) + ssl
                        ot = osb.tile([P, D], f16, tag="ot")
                        for n in range(D // 512):
                            ps_o = bigps.tile([P, 512], f32, tag="bigps",
                                              name=f"ops{ss}_{n}")
                            for hc in range(HC):
                                nc.tensor.matmul(
                                    ps_o,
                                    attnT_sb[:, hc, ss, :],
                                    wo_sb[:, hc, n * 512:(n + 1) * 512],
                                    start=(hc == 0),
                                    stop=(hc == HC - 1))
                            # balance PSUM->SBUF copies across DVE and ACT
                            if n % 4 != 3:
                                nc.vector.tensor_copy(
                                    ot[:, n * 512:(n + 1) * 512], ps_o)
                            else:
                                nc.scalar.copy(
                                    ot[:, n * 512:(n + 1) * 512], ps_o)
                            # final chunk: stream the write per 512-col piece
                            # so the tail isn't one 512KB transfer
                            if sf == SPLIT - 1 and ssl == SQ // P - 1:
                                nc.gpsimd.dma_start(
                                    outp_ap[ss * P:(ss + 1) * P,
                                            n * 512:(n + 1) * 512],
                                    ot[:, n * 512:(n + 1) * 512])
                        if not (sf == SPLIT - 1 and ssl == SQ // P - 1):
                            nc.gpsimd.dma_start(
                                outp_ap[ss * P:(ss + 1) * P], ot)

    nc.compile()
    return nc


def shard_inputs(x, Wqkv, bqkv, Wout):
    """Build the 8 per-core input maps."""
    mask = np.zeros((4, P, 512), np.float16)
    kk = np.arange(P)[:, None]
    qq = np.arange(512)[None, :]
    for j in range(4):
        mask[j] = (qq >= kk + P * j).astype(np.float16)

    # x packed [p, sf, dc, sq] per batch (f16): each per-sf slice is 16KB
    # contiguous per partition
    xps = [np.ascontiguousarray(
        x[b].T.astype(np.float16).reshape(N_DC, P, SPLIT, SQ)
        .transpose(1, 2, 0, 3)) for b in range(B)]
    # Wqkv [D, 3*H*Dh] -> [dc, p, 3, H, dh] (f16)
    W_all = Wqkv.astype(np.float16).reshape(N_DC, P, 3, H, Dh)
    wout_f16 = Wout.astype(np.float16)
    per_hg = {}
    for hg in range(4):
        h0 = hg * HC
        c0 = h0 * Dh
        cw = HC * Dh
        # [p, part, h', dc, dh]: each (part, h') slice is 4KB/partition
        per_hg[hg] = dict(
            wqkv=np.ascontiguousarray(
                W_all[:, :, :, h0:h0 + HC]
                .transpose(1, 2, 3, 0, 4)),
            wout=np.ascontiguousarray(
                wout_f16[c0:c0 + cw].reshape(HC, P, D).transpose(1, 0, 2)),
            bq=np.ascontiguousarray(
                bqkv[c0:c0 + cw].reshape(HC, P).T).astype(np.float32),
            bk=np.ascontiguousarray(
                bqkv[H * Dh + c0:H * Dh + c0 + cw]
                .reshape(HC, P).T).astype(np.float32),
            bv=np.ascontiguousarray(
                bqkv[2 * H * Dh + c0:2 * H * Dh + c0 + cw]
                .reshape(HC, P).T).astype(np.float32),
        )
    in_maps = []
    for c in range(8):
        b, hg = divmod(c, 4)
        g = per_hg[hg]
        in_maps.append({
            "xp": xps[b], "wqkv": g["wqkv"],
            "wout": g["wout"], "bq": g["bq"], "bk": g["bk"], "bv": g["bv"],
            "masks": mask,
        })
    return in_maps


def _prepare():
    """Compile the bass program once and build a cached sharded jit."""
    import jax
    from jax.sharding import Mesh, PartitionSpec
    from jax.experimental.shard_map import shard_map
    from concourse import bass2jax
    from concourse import mybir as mb

    nc = build_program()
    bass2jax.install_neuronx_cc_hook()
    partition_name = (nc.partition_id_tensor.name
                      if nc.partition_id_tensor else None)
    in_names, out_names, out_avals, zero_outs = [], [], [], []
    for alloc in nc.m.functions[0].allocations:
        if not isinstance(alloc, mb.MemoryLocationSet):
            continue
        name = alloc.memorylocations[0].name
        if alloc.kind == "ExternalInput":
            if name != partition_name:
                in_names.append(name)
        elif alloc.kind == "ExternalOutput":
            shape = tuple(alloc.tensor_shape)
            dtype = mb.dt.np(alloc.dtype)
            out_names.append(name)
            out_avals.append(jax.core.ShapedArray(shape, dtype))
            zero_outs.append(np.zeros(shape, dtype))
    n_params, n_outs = len(in_names), len(out_names)
    all_in_names = (in_names + out_names
                    + ([partition_name] if partition_name else []))

    def _body(*args):
        operands = list(args)
        if partition_name is not None:
            operands.append(bass2jax.partition_id_tensor())
        outs = bass2jax._bass_exec_p.bind(
            *operands,
            out_avals=tuple(out_avals),
            in_names=tuple(all_in_names),
            out_names=tuple(out_names),
            lowering_input_output_aliases=(),
            sim_require_finite=True,
            sim_require_nnan=True,
            nc=nc,
        )
        return tuple(outs)

    n_cores = 8
    devices = jax.devices()[:n_cores]
    mesh = Mesh(np.asarray(devices), ("core",))
    sharded = jax.jit(
        shard_map(_body, mesh=mesh,
                  in_specs=(PartitionSpec("core"),) * (n_params + n_outs),
                  out_specs=(PartitionSpec("core"),) * n_outs,
                  check_rep=False),
        donate_argnums=tuple(range(n_params, n_params + n_outs)),
        keep_unused=True,
    )
    return dict(nc=nc, sharded=sharded, in_names=in_names,
                zero_outs=zero_outs, n_cores=n_cores)


def kernel(x, Wqkv, bqkv, Wout, bout):
    import jax

    x = np.asarray(x, dtype=np.float32)
    Wqkv = np.asarray(Wqkv, dtype=np.float32)
    bqkv = np.asarray(bqkv, dtype=np.float32)
    Wout = np.asarray(Wout, dtype=np.float32)
    bout = np.asarray(bout, dtype=np.float32)

    if "ctx" not in _COMPILED:
        _COMPILED["ctx"] = _prepare()
        _COMPILED["nc"] = _COMPILED["ctx"]["nc"]
    ctx = _COMPILED["ctx"]
    n_cores = ctx["n_cores"]

    in_maps = shard_inputs(x, Wqkv, bqkv, Wout)
    per_core = [[np.asarray(m[nm]) for nm in ctx["in_names"]]
                for m in in_maps]
    concat_in = [np.concatenate([per_core[c][i] for c in range(n_cores)],
                                axis=0)
                 for i in range(len(ctx["in_names"]))]
    zs = [np.zeros((n_cores * z.shape[0], *z.shape[1:]), z.dtype)
          for z in ctx["zero_outs"]]
    outs = ctx["sharded"](*concat_in, *zs)
    jax.block_until_ready(outs)
    outp = np.asarray(outs[0])  # [8*S, D] f16, core-major

    out = np.empty((B, S, D), np.float32)
    for b in range(B):
        acc = outp[4 * b * S:(4 * b + 1) * S].astype(np.float32)
        for c in range(4 * b + 1, 4 * b + 4):
            acc += outp[c * S:(c + 1) * S].astype(np.float32)
        out[b] = acc + bout[None, :]
    return out


# revision 15
# speedup vs baseline: 4.4386x; 1.0016x over previous
"""Chunkwise causal attention (full causal MHA + QKV/out projections) on 8 trn2 cores.

v6: all-f16 datapath (f32 PSUM accumulation), PE transposes (XBAR DMA
transpose corrupts on HW despite passing CoreSim), consumption-ordered
startup DMAs (w blocks h-major on the SWDGE rings, masks/wout deferred,
sf0 x in dc-quarters), x prefetch emitted right after Phase A, v-bias
folded into the PSUM drain.

Sharding: data-parallel over batch (B=2) x tensor-parallel over heads (16 -> 4 per
core). Host sums the 4 partial out-projections per batch and adds bout.

Self-contained: hardcodes all shapes from the problem spec.
"""

import numpy as np

import concourse.bass as bass
import concourse.mybir as mybir
import concourse.tile as tile
from concourse import bacc
from concourse.masks import make_identity

# Problem shapes
B, S, D = 2, 2048, 2048
H, Dh = 16, 128
HC = 4                      # heads per core
P = 128
SPLIT = 4
SQ = S // SPLIT             # 512 queries per outer phase
N_DC = D // P               # 16 contraction chunks for projections
N_SC = S // P               # 16 key chunks
SCALE = 1.0 / float(np.sqrt(Dh))
VW = 132                    # vaug row width (129 used: 128 dh + 1 ones col)

f32 = mybir.dt.float32
f16 = mybir.dt.float16

_COMPILED = {}

CFG = dict(XB=2, QB=2, VB=2, EXPB=20, SPB=4, OSB=3, BIGPS=5, PVB=2, TRB=1)


def build_program():
    nc = bacc.Bacc("TRN2", target_bir_lowering=False, debug=False)

    xp = nc.dram_tensor("xp", (P, SPLIT, N_DC, SQ), f16, kind="ExternalInput")
    wqkv = nc.dram_tensor("wqkv", (P, 3, HC, N_DC, P), f16, kind="ExternalInput")
    wout = nc.dram_tensor("wout", (P, HC, D), f16, kind="ExternalInput")
    bq = nc.dram_tensor("bq", (P, HC), f32, kind="ExternalInput")
    bk = nc.dram_tensor("bk", (P, HC), f32, kind="ExternalInput")
    bv = nc.dram_tensor("bv", (P, HC), f32, kind="ExternalInput")
    masks = nc.dram_tensor("masks", (4, P, 512), f16, kind="ExternalInput")
    outp = nc.dram_tensor("outp", (S, D), f16, kind="ExternalOutput")

    xp_ap, wqkv_ap, wout_ap, masks_ap, outp_ap = (
        xp.ap(), wqkv.ap(), wout.ap(), masks.ap(), outp.ap())

    with tile.TileContext(nc, trace_sim=CFG.get('TRACE', False)) as tc:
        with tc.tile_pool(name="const", bufs=1) as const, \
             tc.tile_pool(name="persist", bufs=1) as persist:

            ident = const.tile([P, P], f32, tag="ident")
            make_identity(nc, ident)
            ident16 = const.tile([P, P], f16, tag="ident16")
            nc.scalar.copy(ident16, ident)
            masks_sb = const.tile([P, 4, 512], f16, tag="masks")
            bq_sb = const.tile([P, HC], f32, tag="bq")
            bk_sb = const.tile([P, HC], f32, tag="bk")
            bv_sb = const.tile([P, HC], f32, tag="bv")

            w_sb = persist.tile([P, 3, HC, N_DC, P], f16, tag="w")
            wo_sb = persist.tile([P, HC, D], f16, tag="wo")
            kT_sb = persist.tile([P, HC, S], f16, tag="kT")
            vaug_sb = persist.tile([P, HC, N_SC, VW], f16, tag="vaug")
            attnT_sb = persist.tile([P, HC, N_SC, P], f16, tag="attnT")
            # ones column for softmax denominators
            nc.vector.memset(vaug_sb[:, :, :, Dh:Dh + 1], 1.0)

            with tc.tile_pool(name="xpool", bufs=CFG["XB"]) as xpool, \
                 tc.tile_pool(name="qpool", bufs=CFG["QB"]) as qpool, \
                 tc.tile_pool(name="vpool", bufs=CFG["VB"]) as vpool, \
                 tc.tile_pool(name="exppool", bufs=CFG["EXPB"]) as exppool, \
                 tc.tile_pool(name="spool", bufs=CFG["SPB"]) as spool, \
                 tc.tile_pool(name="osb", bufs=CFG["OSB"]) as osb, \
                 tc.tile_pool(name="bigps", bufs=CFG["BIGPS"], space="PSUM") as bigps, \
                 tc.tile_pool(name="pvps", bufs=CFG["PVB"], space="PSUM") as pvps, \
                 tc.tile_pool(name="trps", bufs=CFG["TRB"], space="PSUM") as trps:

                # ---- startup DMAs, ordered by first use ----
                # sf0 x + tiny biases on the SP/HWDGE rings; all bulk
                # (w/masks/wout/prefetch/output) on the Pool SWDGE rings.
                # sf0 x in 2-dc slivers: the first chain's dc0 gate is a
                # 256KB transfer, not 512KB, under fair-share DMA service
                xT0 = xpool.tile([P, N_DC, SQ], f16, tag="xT")
                for qtr in range(8):
                    nc.sync.dma_start(xT0[:, 2 * qtr:2 * qtr + 2],
                                      xp_ap[:, 0, 2 * qtr:2 * qtr + 2])
                nc.sync.dma_start(bq_sb[:], bq.ap())
                nc.sync.dma_start(bk_sb[:], bk.ap())
                nc.sync.dma_start(bv_sb[:], bv.ap())
                # w blocks in consumption order (h-major); h0's blocks in
                # dc-halves so the first chains aren't gated on full blocks
                for c in range(3 * HC):
                    h, pi = divmod(c, 3)
                    if h == 0:
                        nc.gpsimd.dma_start(w_sb[:, pi, h, 0:N_DC // 2],
                                            wqkv_ap[:, pi, h, 0:N_DC // 2])
                        nc.gpsimd.dma_start(w_sb[:, pi, h, N_DC // 2:],
                                            wqkv_ap[:, pi, h, N_DC // 2:])
                    else:
                        nc.gpsimd.dma_start(w_sb[:, pi, h], wqkv_ap[:, pi, h])
                for j in range(4):
                    nc.gpsimd.dma_start(masks_sb[:, j], masks_ap[j])
                for hc in range(HC):
                    nc.gpsimd.dma_start(wo_sb[:, hc], wout_ap[:, hc])

                for sf in range(SPLIT):
                    # ---- Phase A: qkv projection for this 512-query chunk ----
                    if sf == 0:
                        xT_sb = xT0
                    else:
                        xT_sb = xT_next  # noqa: F821 (prefetched below)
                    qT_sb = qpool.tile([P, HC, SQ], f16, tag="qT")
                    vT_sb = vpool.tile([P, HC, SQ], f16, tag="vT")

                    for h in range(HC):
                        for pi, part in enumerate(("q", "k", "v")):
                            ps_t = bigps.tile([P, 512], f32, tag="bigps",
                                              name=f"proj{sf}_{h}_{part}")
                            for dc in range(N_DC):
                                nc.tensor.matmul(
                                    ps_t,
                                    w_sb[:, pi, h, dc],
                                    xT_sb[:, dc],
                                    start=(dc == 0), stop=(dc == N_DC - 1))
                            if part == "q":
                                nc.vector.tensor_scalar_add(
                                    qT_sb[:, h], ps_t, bq_sb[:, h:h + 1])
                            elif part == "k":
                                nc.vector.tensor_scalar_add(
                                    kT_sb[:, h, sf * SQ:(sf + 1) * SQ],
                                    ps_t, bk_sb[:, h:h + 1])
                            else:
                                # v bias folded into the PSUM drain
                                nc.vector.tensor_scalar_add(
                                    vT_sb[:, h], ps_t, bv_sb[:, h:h + 1])

                    # v: transpose to natural [key, dh] layout on PE
                    for h in range(HC):
                        for scl in range(SQ // P):
                            sc = sf * (SQ // P) + scl
                            tp = trps.tile([P, P], f16, tag="tr")
                            nc.tensor.transpose(
                                tp, vT_sb[:, h, scl * P:(scl + 1) * P], ident16)
                            nc.vector.tensor_copy(
                                vaug_sb[:, h, sc, 0:Dh], tp)

                    # prefetch next x chunk on the SWDGE rings
                    if sf < SPLIT - 1:
                        xT_next = xpool.tile([P, N_DC, SQ], f16, tag="xT")
                        for qtr in range(4):
                            nc.gpsimd.dma_start(
                                xT_next[:, 4 * qtr:4 * qtr + 4],
                                xp_ap[:, sf + 1, 4 * qtr:4 * qtr + 4])

                    # ---- Phase B: attention for the 512 queries of this chunk ----
                    q0 = sf * SQ
                    nk = (q0 + 512) // P        # key chunks needed (causal)
                    for h in range(HC):
                        exps = []
                        for kc in range(nk):
                            j = kc - (nk - 4)
                            # columns < j*128 of this chunk are fully masked
                            off = 0 if j < 1 else j * P
                            qk = bigps.tile([P, 512], f32, tag="bigps",
                                            name=f"qk{sf}_{h}_{kc}")
                            nc.tensor.matmul(
                                qk[:, 0:512 - off],
                                kT_sb[:, h, kc * P:(kc + 1) * P],
                                qT_sb[:, h, off:512],
                                start=True, stop=True)
                            ex = exppool.tile([P, 512], f16, tag="exp")
                            nc.scalar.activation(
                                ex[:, off:512], qk[:, 0:512 - off],
                                mybir.ActivationFunctionType.Exp,
                                scale=SCALE)
                            if j >= 0:
                                nc.vector.tensor_mul(
                                    ex[:, off:512], ex[:, off:512],
                                    masks_sb[:, j, off:512])
                            exps.append(ex)
                        for sub in range(4):
                            nkq = sf * 4 + sub + 1
                            ps = pvps.tile([P, VW], f32, tag="pv")
                            for kc in range(nkq):
                                nc.tensor.matmul(
                                    ps[:, 0:Dh + 1],
                                    exps[kc][:, sub * P:(sub + 1) * P],
                                    vaug_sb[:, h, kc, 0:Dh + 1],
                                    start=(kc == 0),
                                    stop=(kc == nkq - 1))
                            rc = spool.tile([P, 1], f32, tag="rc")
                            nc.vector.reciprocal(rc, ps[:, Dh:Dh + 1])
                            at = spool.tile([P, P], f16, tag="at")
                            nc.vector.tensor_mul(
                                at, ps[:, 0:Dh], rc.to_broadcast((P, P)))
                            tp = trps.tile([P, P], f16, tag="tr")
                            nc.tensor.transpose(tp, at, ident16)
                            nc.vector.tensor_copy(
                                attnT_sb[:, h, sf * 4 + sub, :], tp)

                    # ---- Phase C: partial out projection for this chunk ----
                    for ssl in range(SQ // P):
                        ss = sf * (SQ // P) + ssl
                        ot = osb.tile([P, D], f16, tag="ot")
                        for n in range(D // 512):
                            ps_o = bigps.tile([P, 512], f32, tag="bigps",
                                              name=f"ops{ss}_{n}")
                            for hc in range(HC):
                                nc.tensor.matmul(
                                    ps_o,
                                    attnT_sb[:, hc, ss, :],
                                    wo_sb[:, hc, n * 512:(n + 1) * 512],
                                    start=(hc == 0),
                                    stop=(hc == HC - 1))
                            # balance PSUM->SBUF copies across DVE and ACT
                            if n % 4 != 3:
                                nc.vector.tensor_copy(
                                    ot[:, n * 512:(n + 1) * 512], ps_o)
                            else:
                                nc.scalar.copy(
                                    ot[:, n * 512:(n + 1) * 512], ps_o)
                            # final chunk: stream the write per 512-col piece
                            # so the tail isn't one 512KB transfer
                            if sf == SPLIT - 1 and ssl == SQ // P - 1:
                                nc.gpsimd.dma_start(
                                    outp_ap[ss * P:(ss + 1) * P,
                                            n * 512:(n + 1) * 512],
                                    ot[:, n * 512:(n + 1) * 512])
                        if not (sf == SPLIT - 1 and ssl == SQ // P - 1):
                            nc.gpsimd.dma_start(
                                outp_ap[ss * P:(ss + 1) * P], ot)

    nc.compile()
    return nc


def shard_inputs(x, Wqkv, bqkv, Wout):
    """Build the 8 per-core input maps."""
    mask = np.zeros((4, P, 512), np.float16)
    kk = np.arange(P)[:, None]
    qq = np.arange(512)[None, :]
    for j in range(4):
        mask[j] = (qq >= kk + P * j).astype(np.float16)

    # x packed [p, sf, dc, sq] per batch (f16): each per-sf slice is 16KB
    # contiguous per partition
    xps = [np.ascontiguousarray(
        x[b].T.astype(np.float16).reshape(N_DC, P, SPLIT, SQ)
        .transpose(1, 2, 0, 3)) for b in range(B)]
    # Wqkv [D, 3*H*Dh] -> [dc, p, 3, H, dh] (f16)
    W_all = Wqkv.astype(np.float16).reshape(N_DC, P, 3, H, Dh)
    wout_f16 = Wout.astype(np.float16)
    per_hg = {}
    for hg in range(4):
        h0 = hg * HC
        c0 = h0 * Dh
        cw = HC * Dh
        # [p, part, h', dc, dh]: each (part, h') slice is 4KB/partition
        per_hg[hg] = dict(
            wqkv=np.ascontiguousarray(
                W_all[:, :, :, h0:h0 + HC]
                .transpose(1, 2, 3, 0, 4)),
            wout=np.ascontiguousarray(
                wout_f16[c0:c0 + cw].reshape(HC, P, D).transpose(1, 0, 2)),
            bq=np.ascontiguousarray(
                bqkv[c0:c0 + cw].reshape(HC, P).T).astype(np.float32),
            bk=np.ascontiguousarray(
                bqkv[H * Dh + c0:H * Dh + c0 + cw]
                .reshape(HC, P).T).astype(np.float32),
            bv=np.ascontiguousarray(
                bqkv[2 * H * Dh + c0:2 * H * Dh + c0 + cw]
                .reshape(HC, P).T).astype(np.float32),
        )
    in_maps = []
    for c in range(8):
        b, hg = divmod(c, 4)
        g = per_hg[hg]
        in_maps.append({
            "xp": xps[b], "wqkv": g["wqkv"],
            "wout": g["wout"], "bq": g["bq"], "bk": g["bk"], "bv": g["bv"],
            "masks": mask,
        })
    return in_maps


def _prepare():
    """Compile the bass program once and build a cached sharded jit."""
    import jax
    from jax.sharding import Mesh, PartitionSpec
    from jax.experimental.shard_map import shard_map
    from concourse import bass2jax
    from concourse import mybir as mb

    nc = build_program()
    bass2jax.install_neuronx_cc_hook()
    partition_name = (nc.partition_id_tensor.name
                      if nc.partition_id_tensor else None)
    in_names, out_names, out_avals, zero_outs = [], [], [], []
    for alloc in nc.m.functions[0].allocations:
        if not isinstance(alloc, mb.MemoryLocationSet):
            continue
        name = alloc.memorylocations[0].name
        if alloc.kind == "ExternalInput":
            if name != partition_name:
                in_names.append(name)
        elif alloc.kind == "ExternalOutput":
            shape = tuple(alloc.tensor_shape)
            dtype = mb.dt.np(alloc.dtype)
            out_names.append(name)
            out_avals.append(jax.core.ShapedArray(shape, dtype))
            zero_outs.append(np.zeros(shape, dtype))
    n_params, n_outs = len(in_names), len(out_names)
    all_in_names = (in_names + out_names
                    + ([partition_name] if partition_name else []))

    def _body(*args):
        operands = list(args)
        if partition_name is not None:
            operands.append(bass2jax.partition_id_tensor())
        outs = bass2jax._bass_exec_p.bind(
            *operands,
            out_avals=tuple(out_avals),
            in_names=tuple(all_in_names),
            out_names=tuple(out_names),
            lowering_input_output_aliases=(),
            sim_require_finite=True,
            sim_require_nnan=True,
            nc=nc,
        )
        return tuple(outs)

    n_cores = 8
    devices = jax.devices()[:n_cores]
    mesh = Mesh(np.asarray(devices), ("core",))
    sharded = jax.jit(
        shard_map(_body, mesh=mesh,
                  in_specs=(PartitionSpec("core"),) * (n_params + n_outs),
                  out_specs=(PartitionSpec("core"),) * n_outs,
                  check_rep=False),
        donate_argnums=tuple(range(n_params, n_params + n_outs)),
        keep_unused=True,
    )
    return dict(nc=nc, sharded=sharded, in_names=in_names,
                zero_outs=zero_outs, n_cores=n_cores)


def kernel(x, Wqkv, bqkv, Wout, bout):
    import jax

    x = np.asarray(x, dtype=np.float32)
    Wqkv = np.asarray(Wqkv, dtype=np.float32)
    bqkv = np.asarray(bqkv, dtype=np.float32)
    Wout = np.asarray(Wout, dtype=np.float32)
    bout = np.asarray(bout, dtype=np.float32)

    if "ctx" not in _COMPILED:
        _COMPILED["ctx"] = _prepare()
        _COMPILED["nc"] = _COMPILED["ctx"]["nc"]
    ctx = _COMPILED["ctx"]
    n_cores = ctx["n_cores"]

    in_maps = shard_inputs(x, Wqkv, bqkv, Wout)
    per_core = [[np.asarray(m[nm]) for nm in ctx["in_names"]]
                for m in in_maps]
    concat_in = [np.concatenate([per_core[c][i] for c in range(n_cores)],
                                axis=0)
                 for i in range(len(ctx["in_names"]))]
    zs = [np.zeros((n_cores * z.shape[0], *z.shape[1:]), z.dtype)
          for z in ctx["zero_outs"]]
    outs = ctx["sharded"](*concat_in, *zs)
    jax.block_until_ready(outs)
    outp = np.asarray(outs[0])  # [8*S, D] f16, core-major

    out = np.empty((B, S, D), np.float32)
    for b in range(B):
        acc = outp[4 * b * S:(4 * b + 1) * S].astype(np.float32)
        for c in range(4 * b + 1, 4 * b + 4):
            acc += outp[c * S:(c + 1) * S].astype(np.float32)
        out[b] = acc + bout[None, :]
    return out
